# revision 17
# baseline (speedup 1.0000x reference)
"""2-layer GAT (graph attention) on 8 Trainium2 NeuronCores.

Sharding: query/node rows split 8 ways (512 rows per core). Attention scores
are computed transposed ([key_block=128, q=512]) so probability tiles feed the
TensorE directly as lhsT.

Key trick vs the straightforward version: the per-element exp is factored
away. With z = e_src[q] + e_dst[k],
    exp(leakyrelu(z)) = max(exp(z), exp(alpha*z))
                      = exp(e_src[q]) * exp(e_dst[k]) * max(1, exp(-(1-a)z))
The exp(e_src[q]) factor is constant per attention-softmax row and cancels,
so the unnormalised weight is  w[k,q] = mask * max(1, G[k]*TH[q])  with
G = exp(-(1-a)e_dst), TH = exp(-(1-a)e_src), and the exp(e_dst[k]) factor is
folded into the value rows (hhat = exp(e_dst)*[h | 1]) before the AllGather.
Scores therefore need only two cheap elementwise passes per tile: a DVE
tensor_scalar (mult+max, 4x bf16 mode) and a DVE tensor_tensor mask multiply
(2x mode) - no ScalarE exp over the N^2 score matrix at all. A few groups are
routed to ScalarE(Relu)+Pool to balance engines.

G for all nodes is shipped in a tiny f32 AllGather that fires before the big
h AllGathers, so all score elementwise work is gather-independent and
overlaps the collectives. Same for layer 2 (separate tiny G2 gather).
"""

import numpy as np
import ml_dtypes

import concourse.bass as bass
import concourse.tile as tile
from concourse import bacc, mybir
from concourse.bass_utils import run_bass_kernel_spmd
from concourse.masks import make_identity

P = 128
N, F, O, H, C = 4096, 512, 256, 4, 16
NCORES = 8
Q = N // NCORES          # 512 query rows per core
QC = Q // P              # 4 query chunks
MB = N // P              # 32 key blocks
KB = F // P              # 4 contraction blocks over F
OB = O // P              # 2 contraction blocks over O
ALPHA = 0.2
BETA = -(1.0 - ALPHA)    # -0.8: exponent scale for the G/TH factors
PAY = C + 2              # w2p cols: 0:16 W2, 16 e2dst vec, 17 e2src vec
PAYG = C + 1             # layer-2 payload cols: 0:16 hhat2, 16 A2
HC = O + 1               # per-head gathered cols: hhat | A
HCH = 2 * HC             # 514 per gather half (2 heads)
# Layer-1 score weights and values are globally scaled by 2^-6 / 2^-3 (folded
# into the G and A exponentials; both cancel in the softmax ratio) so late
# groups can drop to fp8e4m3 and use DoubleRow matmuls (2 key blocks per PE
# instruction). Those groups' mask-multiplies run on the Pool engine (emitted
# after the AllGathers have drained its queue), writing fp8 directly.
LN2 = 0.6931471805599453
SCL1 = 2.0 ** -6             # pm scale (fold into G)
SCLH = 2.0 ** -3             # hhat scale (fold into A)
DR_AFTER_L1 = 26             # groups (hp*16+mbp) >= this: fp8 DoubleRow + Pool
POOL_AFTER_L2 = 11           # of 16 L2 groups: mask-mult on Pool
PM_BUFS = 7
ZT_BUFS = 6
NO_COLL = False              # timing probe: skip AllGathers (results invalid)
NO_MASK = False              # timing probe: skip mask multiplies (results invalid)

bf16 = mybir.dt.bfloat16
f32 = mybir.dt.float32
fp8 = mybir.dt.float8e4
AF = mybir.ActivationFunctionType
ALU = mybir.AluOpType
AX = mybir.AxisListType


def _allgather(nc, in_d, out_d, nrows):
    if NO_COLL:
        # timing probe: fake the gather with a local row-copy (wrong data)
        nc.sync.dma_start(out_d[0:nrows, :], in_d[:])
        return
    nc.gpsimd.collective_compute(
        "AllGather", ALU.bypass,
        replica_groups=[list(range(NCORES))],
        ins=[in_d.opt()], outs=[out_d.opt()])


def _build(reps=1):
    nc = bacc.Bacc("TRN2", target_bir_lowering=False, debug=False,
                   num_devices=NCORES)

    xTq_d = nc.dram_tensor("xTq", [F, Q], bf16, kind="ExternalInput").ap()
    maskT_d = nc.dram_tensor("maskT", [N, Q], bf16, kind="ExternalInput").ap()
    w1_d = nc.dram_tensor("w1", [F, H * O], bf16, kind="ExternalInput").ap()
    wsd_d = nc.dram_tensor("wsd", [F, 2 * H], bf16, kind="ExternalInput").ap()
    w2p_d = nc.dram_tensor("w2p", [O, PAY], bf16, kind="ExternalInput").ap()
    out_d = nc.dram_tensor("out", [Q, C], f32, kind="ExternalOutput").ap()

    with tile.TileContext(nc) as tc:
        for _ in range(reps):
            _emit(tc, xTq_d, maskT_d, w1_d, wsd_d, w2p_d, out_d)
    nc.compile()
    return nc


def _emit(tc, xTq_d, maskT_d, w1_d, wsd_d, w2p_d, out_d):
    nc = tc.nc
    with tc.tile_pool(name="singles", bufs=1) as singles:
        # ---- persistent SBUF tensors ----
        xTq_sb = singles.tile([P, KB, Q], bf16)
        maskT_sb = singles.tile([P, MB, Q], bf16)
        w1_sb = singles.tile([P, KB, H * O], bf16)
        wsd_sb = singles.tile([P, KB, 2 * H], bf16)
        w2p_sb = singles.tile([P, OB, PAY], bf16)
        ones1 = singles.tile([1, P], f32)
        ident = singles.tile([P, P], bf16)
        TH_sb = singles.tile([P, H, Q], bf16)     # exp(BETA*e_src) bcast
        Eown_sb = singles.tile([P, QC, 2 * H], f32)
        Aown_sb = singles.tile([P, H, QC], f32)   # exp(e_dst) own rows
        gpay_sb = singles.tile([P, QC, H], f32)   # exp(BETA*e_dst) own rows
        G_sb = singles.tile([P, MB, H], f32)      # gathered G, all nodes
        h_sbA = singles.tile([P, MB, HCH], bf16)  # heads 0-1 [hhat|A]
        h_sbB = singles.tile([P, MB, HCH], bf16)  # heads 2-3
        h8_sbB = singles.tile([P, MB, HCH], fp8)  # fp8 cast of late h_sbB
        hpayA_sb = singles.tile([P, QC, HCH], bf16)
        hpayB_sb = singles.tile([P, QC, HCH], bf16)
        x2acc = singles.tile([P, QC, O], f32)
        x2bf = singles.tile([P, QC, O], bf16)
        x2T = singles.tile([P, OB, Q], bf16)
        e2d_own = singles.tile([P, QC], f32)
        A2_sb = singles.tile([P, QC], f32)
        g2own_sb = singles.tile([P, QC], f32)
        pay_sb = singles.tile([P, QC, PAYG], bf16)
        h2g_sb = singles.tile([P, MB, PAYG], bf16)
        TH2_sb = singles.tile([P, Q], bf16)
        G2_all = singles.tile([P, MB], f32)

        # ---- input DMAs (order: smallest/earliest-needed first) ----
        xTq_r = xTq_d.rearrange("(kb p) q -> p kb q", p=P)
        for kb in range(KB):
            nc.sync.dma_start(xTq_sb[:, kb, :], xTq_r[:, kb, :])
        nc.sync.dma_start(wsd_sb[:], wsd_d.rearrange("(kb p) c -> p kb c", p=P))
        nc.sync.dma_start(w2p_sb[:], w2p_d.rearrange("(ob p) c -> p ob c", p=P))
        w1_r = w1_d.rearrange("(kb p) c -> p kb c", p=P)
        for half in range(2):
            s = bass.ds(half * (H * O // 2), H * O // 2)
            nc.sync.dma_start(w1_sb[:, :, s], w1_r[:, :, s])
        maskT_r = maskT_d.rearrange("(b p) q -> p b q", p=P)
        for g in range(8):
            s = bass.ts(g, MB // 8)
            nc.sync.dma_start(maskT_sb[:, s, :], maskT_r[:, s, :])

        nc.vector.memset(ones1[:], 1.0)
        bG = singles.tile([P, 1], f32)
        bA = singles.tile([P, 1], f32)
        nc.vector.memset(bG[:], -6.0 * LN2)   # G *= 2^-6
        nc.vector.memset(bA[:], -3.0 * LN2)   # A *= 2^-3
        make_identity(nc, ident[:])

        # ---- phase B: own-row E, tiny G gather, TH tiles ----
        with tc.tile_pool(name="b_psum", bufs=1, space="PSUM") as bpp, \
             tc.tile_pool(name="b_sb", bufs=2) as bsb, \
             tc.tile_pool(name="g_dram", bufs=1, space="DRAM") as gdram:
            for qc in range(QC):
                ps_E = bpp.tile([P, 2 * H], f32, tag="ps_E")
                for kb in range(KB):
                    nc.tensor.matmul(ps_E[:], xTq_sb[:, kb, bass.ts(qc, P)],
                                     wsd_sb[:, kb, :],
                                     start=(kb == 0), stop=(kb == KB - 1))
                nc.scalar.activation(Eown_sb[:, qc, :], ps_E[:], AF.Copy)
            for h in range(H):
                nc.scalar.activation(gpay_sb[:, :, h], Eown_sb[:, :, H + h],
                                     AF.Exp, scale=BETA, bias=bG[:])
            gpay_d = gdram.tile([Q, H], f32)
            ggath_d = gdram.tile([N, H], f32, addr_space="Shared")
            nc.sync.dma_start(gpay_d.rearrange("(qc p) c -> p qc c", p=P),
                              gpay_sb[:])
            _allgather(nc, gpay_d, ggath_d, Q)
            nc.sync.dma_start(G_sb[:],
                              ggath_d.rearrange("(b p) c -> p b c", p=P))
            for h in range(H):
                nc.scalar.activation(Aown_sb[:, h, :], Eown_sb[:, :, H + h],
                                     AF.Exp, bias=bA[:])
            # TH per head: esrcT row -> broadcast -> exp(BETA*.)
            for h in range(H):
                ps_es = bpp.tile([1, Q], f32, tag="ps_es")
                for kb in range(KB):
                    nc.tensor.matmul(ps_es[:], wsd_sb[:, kb, h:h + 1],
                                     xTq_sb[:, kb, :],
                                     start=(kb == 0), stop=(kb == KB - 1))
                esrcT = bsb.tile([1, Q], f32, tag="esrcT")
                nc.scalar.activation(esrcT[:], ps_es[:], AF.Copy)
                ps_b = bpp.tile([P, Q], f32, tag="ps_b")
                nc.tensor.matmul(ps_b[:], ones1[:], esrcT[:],
                                 start=True, stop=True)
                nc.scalar.activation(TH_sb[:, h, :], ps_b[:], AF.Exp,
                                     scale=BETA)

        # ---- phase C: hhat for own rows, 2-stage AllGather ----
        with tc.tile_pool(name="h_psum", bufs=2, space="PSUM") as hpp, \
             tc.tile_pool(name="h_dram", bufs=1, space="DRAM") as hdram:
            for qc in range(QC):
                ps_h = hpp.tile([P, H * O], f32, tag="ps_h")
                for half in range(2):
                    s = bass.ds(half * 512, 512)
                    for kb in range(KB):
                        nc.tensor.matmul(ps_h[:, s],
                                         xTq_sb[:, kb, bass.ts(qc, P)],
                                         w1_sb[:, kb, s],
                                         start=(kb == 0), stop=(kb == KB - 1))
                for h in range(H):
                    dst_sb = hpayA_sb if h < 2 else hpayB_sb
                    c0 = (h % 2) * HC
                    nc.scalar.activation(dst_sb[:, qc, c0:c0 + O],
                                         ps_h[:, bass.ts(h, O)], AF.Copy,
                                         scale=Aown_sb[:, h, qc:qc + 1])
                    nc.vector.tensor_copy(dst_sb[:, qc, c0 + O:c0 + O + 1],
                                          Aown_sb[:, h, qc:qc + 1])
            hpayA_d = hdram.tile([Q, HCH], bf16)
            hgathA_d = hdram.tile([N, HCH], bf16, addr_space="Shared")
            hpayB_d = hdram.tile([Q, HCH], bf16)
            hgathB_d = hdram.tile([N, HCH], bf16, addr_space="Shared")
            nc.sync.dma_start(hpayA_d.rearrange("(qc p) c -> p qc c", p=P),
                              hpayA_sb[:])
            nc.sync.dma_start(hpayB_d.rearrange("(qc p) c -> p qc c", p=P),
                              hpayB_sb[:])
            _allgather(nc, hpayA_d, hgathA_d, Q)
            _allgather(nc, hpayB_d, hgathB_d, Q)
            hgA_r = hgathA_d.rearrange("(b p) c -> p b c", p=P)
            hgB_r = hgathB_d.rearrange("(b p) c -> p b c", p=P)
            for g in range(8):
                s = bass.ts(g, MB // 8)
                nc.sync.dma_start(h_sbA[:, s, :], hgA_r[:, s, :])
                nc.sync.dma_start(h_sbB[:, s, :], hgB_r[:, s, :])

        # ---- phase D: layer-1 attention, head-pair outer ----
        with tc.tile_pool(name="acc_psum", bufs=1, space="PSUM") as accp, \
             tc.tile_pool(name="pm_pool", bufs=PM_BUFS) as pm_pool, \
             tc.tile_pool(name="pm8_pool", bufs=3) as pm8_pool, \
             tc.tile_pool(name="zt_pool", bufs=ZT_BUFS) as zt_pool, \
             tc.tile_pool(name="small1", bufs=4) as sp1:
            for hp in range(2):
                h_half = h_sbA if hp == 0 else h_sbB
                accs = {}
                for hh in range(2):
                    for qc in range(QC):
                        accs[(hh, qc)] = accp.tile(
                            [P, HC], f32, tag=f"acc{hh}_{qc}",
                            name=f"acc{hh}_{qc}")
                for mbp in range(MB // 2):
                    route_dr = (hp * (MB // 2) + mbp) >= DR_AFTER_L1
                    zt = zt_pool.tile([P, 2, 2, Q], bf16, tag="zt", name="zt")
                    for mbi in range(2):
                        mb = mbp * 2 + mbi
                        for hh in range(2):
                            h = hp * 2 + hh
                            # t = max(G*TH, 2^-6) on DVE (4x ts mode);
                            # the 2^-6 pm scale rides in G
                            nc.vector.tensor_scalar(
                                zt[:, mbi, hh, :], TH_sb[:, h, :],
                                G_sb[:, mb, h:h + 1], SCL1,
                                ALU.mult, ALU.max)
                    if route_dr:
                        # fp8 pm via Pool; values feed DoubleRow matmuls
                        pm8 = pm8_pool.tile([P, 2, 2, Q], fp8, tag="pm8",
                                            name="pm8")
                        for hh in range(2):
                            nc.gpsimd.tensor_mul(
                                pm8[:, :, hh, :], zt[:, :, hh, :],
                                maskT_sb[:, mbp * 2:mbp * 2 + 2, :])
                        nc.scalar.activation(
                            h8_sbB[:, mbp * 2:mbp * 2 + 2, :],
                            h_sbB[:, mbp * 2:mbp * 2 + 2, :], AF.Copy)
                        for hh in range(2):
                            rhs8 = h8_sbB[:, mbp * 2:mbp * 2 + 2,
                                          (hh * HC):(hh * HC + HC)]
                            for qc in range(QC):
                                nc.tensor.matmul(
                                    accs[(hh, qc)][:],
                                    pm8[:, :, hh, bass.ts(qc, P)],
                                    rhs8, start=False,
                                    stop=(mbp == MB // 2 - 1),
                                    perf_mode=mybir.MatmulPerfMode.DoubleRow)
                    else:
                        if NO_MASK:
                            pm = zt
                        else:
                            pm = pm_pool.tile([P, 2, 2, Q], bf16, tag="pm",
                                              name="pm")
                            for hh in range(2):
                                nc.vector.tensor_mul(
                                    pm[:, :, hh, :], zt[:, :, hh, :],
                                    maskT_sb[:, mbp * 2:mbp * 2 + 2, :])
                        for mbi in range(2):
                            mb = mbp * 2 + mbi
                            for hh in range(2):
                                rhs = h_half[:, mb, (hh * HC):(hh * HC + HC)]
                                for qc in range(QC):
                                    nc.tensor.matmul(
                                        accs[(hh, qc)][:],
                                        pm[:, mbi, hh, bass.ts(qc, P)],
                                        rhs, start=(mb == 0),
                                        stop=(mb == MB - 1))
                for hh in range(2):
                    h = hp * 2 + hh
                    for qc in range(QC):
                        r = sp1.tile([P, 1], f32, tag="r")
                        nc.vector.reciprocal(r[:], accs[(hh, qc)][:, O:O + 1])
                        nc.vector.tensor_scalar_mul(r[:], r[:], 1.0 / H)
                        if h == 0:
                            nc.scalar.activation(
                                x2acc[:, qc, :], accs[(hh, qc)][:, 0:O],
                                AF.Copy, scale=r[:])
                        else:
                            nc.vector.scalar_tensor_tensor(
                                x2acc[:, qc, :], accs[(hh, qc)][:, 0:O], r[:],
                                x2acc[:, qc, :], op0=ALU.mult, op1=ALU.add)

        # ---- phase E: relu, transpose, layer-2 projections ----
        nc.scalar.activation(x2bf[:], x2acc[:], AF.Relu)
        with tc.tile_pool(name="l2_psum", bufs=2, space="PSUM") as lpp, \
             tc.tile_pool(name="l2_sb", bufs=2) as lsb, \
             tc.tile_pool(name="g2_dram", bufs=1, space="DRAM") as g2dram:
            for qc in range(QC):
                for ob in range(OB):
                    tp = lpp.tile([P, P], bf16, tag="tp")
                    nc.tensor.transpose(tp[:], x2bf[:, qc, bass.ts(ob, P)],
                                        ident[:])
                    nc.scalar.activation(x2T[:, ob, bass.ts(qc, P)], tp[:],
                                         AF.Copy)
            for qc in range(QC):
                ps2 = lpp.tile([P, PAY], f32, tag="ps2")
                for ob in range(OB):
                    nc.tensor.matmul(ps2[:], x2T[:, ob, bass.ts(qc, P)],
                                     w2p_sb[:, ob, :],
                                     start=(ob == 0), stop=(ob == OB - 1))
                nc.vector.tensor_copy(e2d_own[:, qc:qc + 1], ps2[:, C:C + 1])
                nc.scalar.activation(A2_sb[:, qc:qc + 1], ps2[:, C:C + 1],
                                     AF.Exp)
                nc.scalar.activation(g2own_sb[:, qc:qc + 1], ps2[:, C:C + 1],
                                     AF.Exp, scale=BETA)
                nc.scalar.activation(pay_sb[:, qc, 0:C], ps2[:, 0:C], AF.Copy,
                                     scale=A2_sb[:, qc:qc + 1])
                nc.vector.tensor_copy(pay_sb[:, qc, C:C + 1],
                                      A2_sb[:, qc:qc + 1])
            # tiny G2 gather first, then the payload gather
            g2own_d = g2dram.tile([Q, 1], f32)
            g2gath_d = g2dram.tile([N, 1], f32, addr_space="Shared")
            nc.sync.dma_start(g2own_d.rearrange("(qc p) c -> p qc c", p=P),
                              g2own_sb[:, :, None])
            _allgather(nc, g2own_d, g2gath_d, Q)
            nc.sync.dma_start(G2_all[:, :, None],
                              g2gath_d.rearrange("(b p) c -> p b c", p=P))
            # TH2 from e2srcT
            ps_e2 = lpp.tile([1, Q], f32, tag="ps_e2")
            for ob in range(OB):
                nc.tensor.matmul(ps_e2[:], w2p_sb[:, ob, C + 1:C + 2],
                                 x2T[:, ob, :],
                                 start=(ob == 0), stop=(ob == OB - 1))
            e2srcT = lsb.tile([1, Q], f32, tag="e2srcT")
            nc.scalar.activation(e2srcT[:], ps_e2[:], AF.Copy)
            ps_b2 = lpp.tile([P, Q], f32, tag="ps_b2")
            nc.tensor.matmul(ps_b2[:], ones1[:], e2srcT[:],
                             start=True, stop=True)
            nc.scalar.activation(TH2_sb[:], ps_b2[:], AF.Exp, scale=BETA)

        # ---- phase F: AllGather packed payload ----
        with tc.tile_pool(name="dram", bufs=1, space="DRAM") as dram:
            pay_d = dram.tile([Q, PAYG], bf16)
            gath_d = dram.tile([N, PAYG], bf16, addr_space="Shared")
            nc.sync.dma_start(pay_d.rearrange("(qc p) c -> p qc c", p=P),
                              pay_sb[:])
            _allgather(nc, pay_d, gath_d, Q)
            h2g_r = gath_d.rearrange("(b p) c -> p b c", p=P)
            for g in range(4):
                s = bass.ts(g, MB // 4)
                nc.sync.dma_start(h2g_sb[:, s, :], h2g_r[:, s, :])

            # ---- phase G: layer-2 attention ----
            with tc.tile_pool(name="acc2_psum", bufs=1, space="PSUM") as acc2p, \
                 tc.tile_pool(name="pm2_pool", bufs=8) as pm2_pool, \
                 tc.tile_pool(name="zt2_pool", bufs=4) as zt2_pool, \
                 tc.tile_pool(name="small2", bufs=4) as sp2:
                accs2 = []
                for qc in range(QC):
                    accs2.append(acc2p.tile([P, C + 1], f32, tag=f"a2_{qc}",
                                            name=f"a2_{qc}"))
                for mbp in range(MB // 2):
                    route_pool = mbp >= POOL_AFTER_L2
                    zt2 = zt2_pool.tile([P, 2, Q], bf16, tag="zt2", name="zt2")
                    pm2 = pm2_pool.tile([P, 2, Q], bf16, tag="pm2", name="pm2")
                    for mbi in range(2):
                        mb = mbp * 2 + mbi
                        nc.vector.tensor_scalar(
                            zt2[:, mbi, :], TH2_sb[:],
                            G2_all[:, mb:mb + 1], 1.0,
                            ALU.mult, ALU.max)
                    if NO_MASK:
                        pm2 = zt2
                    else:
                        eng = nc.gpsimd if route_pool else nc.vector
                        eng.tensor_mul(
                            pm2[:], zt2[:],
                            maskT_sb[:, mbp * 2:mbp * 2 + 2, :])
                    for mbi in range(2):
                        mb = mbp * 2 + mbi
                        for qc in range(QC):
                            nc.tensor.matmul(accs2[qc][:],
                                             pm2[:, mbi, bass.ts(qc, P)],
                                             h2g_sb[:, mb, 0:C + 1],
                                             start=(mb == 0),
                                             stop=(mb == MB - 1))
                # log-softmax, batched by activation function
                logits_all = sp2.tile([P, QC, C], f32, tag="logits_all",
                                      name="logits_all")
                negmax_all = sp2.tile([P, QC], f32, tag="negmax_all",
                                      name="negmax_all")
                ssum_all = sp2.tile([P, QC], f32, tag="ssum_all",
                                    name="ssum_all")
                lse_all = sp2.tile([P, QC], f32, tag="lse_all", name="lse_all")
                for qc in range(QC):
                    r2 = sp2.tile([P, 1], f32, tag="r2")
                    nc.vector.reciprocal(r2[:], accs2[qc][:, C:C + 1])
                    nc.vector.tensor_scalar_mul(logits_all[:, qc, :],
                                                accs2[qc][:, 0:C], r2[:])
                    nc.vector.reduce_max(negmax_all[:, qc:qc + 1],
                                         logits_all[:, qc, :], axis=AX.X,
                                         negate=True)
                for qc in range(QC):
                    expt = sp2.tile([P, C], f32, tag="expt")
                    nc.scalar.activation(expt[:], logits_all[:, qc, :], AF.Exp,
                                         bias=negmax_all[:, qc:qc + 1],
                                         accum_out=ssum_all[:, qc:qc + 1])
                nc.scalar.activation(lse_all[:], ssum_all[:], AF.Ln)
                for qc in range(QC):
                    res = sp2.tile([P, C], f32, tag="res")
                    nc.vector.tensor_scalar(res[:], logits_all[:, qc, :],
                                            negmax_all[:, qc:qc + 1],
                                            lse_all[:, qc:qc + 1],
                                            ALU.add, ALU.subtract)
                    nc.sync.dma_start(out_d[bass.ts(qc, P), :], res[:])


_CACHED = None


def _get_nc():
    global _CACHED
    if _CACHED is None:
        _CACHED = _build()
    return _CACHED


def kernel(x, adj, W1, a1, W2, a2):
    x = np.asarray(x, dtype=np.float32)
    adj = np.asarray(adj)
    W1 = np.asarray(W1, dtype=np.float32)
    a1 = np.asarray(a1, dtype=np.float32)
    W2 = np.asarray(W2, dtype=np.float32)
    a2 = np.asarray(a2, dtype=np.float32)

    bf = ml_dtypes.bfloat16
    xT = np.ascontiguousarray(x.T).astype(bf)                     # [F, N]
    # fused score weights: e_src = x @ (W1 @ a_src), e_dst likewise
    wsrc = np.einsum("hfo,ho->fh", W1, a1[:, :O])                 # [F, H]
    wdst = np.einsum("hfo,ho->fh", W1, a1[:, O:])                 # [F, H]
    wsd = np.concatenate([wsrc, wdst], axis=1).astype(bf)         # [F, 2H]
    w1cat = np.concatenate([W1[h] for h in range(H)], 1).astype(bf)  # [F, H*O]
    w2p = np.zeros((O, PAY), np.float32)
    w2p[:, 0:C] = W2[0]
    w2p[:, C] = W2[0] @ a2[0, C:]      # e2_dst vector
    w2p[:, C + 1] = W2[0] @ a2[0, :C]  # e2_src vector
    w2p = w2p.astype(bf)

    adj_on = adj > 0
    in_maps = []
    for c in range(NCORES):
        rows = slice(c * Q, (c + 1) * Q)
        in_maps.append({
            "xTq": np.ascontiguousarray(xT[:, rows]),
            "maskT": np.ascontiguousarray(adj_on[rows, :].T).astype(bf),
            "w1": w1cat,
            "wsd": wsd,
            "w2p": w2p,
        })

    nc = _get_nc()
    res = run_bass_kernel_spmd(nc, in_maps, core_ids=list(range(NCORES)))
    return np.concatenate([res.results[c]["out"] for c in range(NCORES)], 0)


# revision 19
# speedup vs baseline: 2.2876x; 2.2876x over previous
"""2-layer GAT (graph attention) on 8 Trainium2 NeuronCores.

Sharding: query/node rows split 8 ways (512 rows per core). Attention scores
are computed transposed ([key_block=128, q=512]) so probability tiles feed the
TensorE directly as lhsT.

Key trick vs the straightforward version: the per-element exp is factored
away. With z = e_src[q] + e_dst[k],
    exp(leakyrelu(z)) = max(exp(z), exp(alpha*z))
                      = exp(e_src[q]) * exp(e_dst[k]) * max(1, exp(-(1-a)z))
The exp(e_src[q]) factor is constant per attention-softmax row and cancels,
so the unnormalised weight is  w[k,q] = mask * max(1, G[k]*TH[q])  with
G = exp(-(1-a)e_dst), TH = exp(-(1-a)e_src), and the exp(e_dst[k]) factor is
folded into the value rows (hhat = exp(e_dst)*[h | 1]) before the AllGather.
Scores therefore need only two cheap elementwise passes per tile: a DVE
tensor_scalar (mult+max, 4x bf16 mode) and a DVE tensor_tensor mask multiply
(2x mode) - no ScalarE exp over the N^2 score matrix at all. A few groups are
routed to ScalarE(Relu)+Pool to balance engines.

G for all nodes is shipped in a tiny f32 AllGather that fires before the big
h AllGathers, so all score elementwise work is gather-independent and
overlaps the collectives. Same for layer 2 (separate tiny G2 gather).
"""

import numpy as np
import ml_dtypes

import concourse.bass as bass
import concourse.tile as tile
from concourse import bacc, mybir
from concourse.bass_utils import run_bass_kernel_spmd
from concourse.masks import make_identity

P = 128
N, F, O, H, C = 4096, 512, 256, 4, 16
NCORES = 8
Q = N // NCORES          # 512 query rows per core
QC = Q // P              # 4 query chunks
MB = N // P              # 32 key blocks
KB = F // P              # 4 contraction blocks over F
OB = O // P              # 2 contraction blocks over O
ALPHA = 0.2
BETA = -(1.0 - ALPHA)    # -0.8: exponent scale for the G/TH factors
PAY = C + 2              # w2p cols: 0:16 W2, 16 e2dst vec, 17 e2src vec
PAYG = C + 1             # layer-2 payload cols: 0:16 hhat2, 16 A2
HC = O + 1               # per-head gathered cols: hhat | A
HCH = 2 * HC             # 514 per gather half (2 heads)
# Layer-1 score weights and values are globally scaled by 2^-6 / 2^-3 (folded
# into the G and A exponentials; both cancel in the softmax ratio) so late
# groups can drop to fp8e4m3 and use DoubleRow matmuls (2 key blocks per PE
# instruction). Those groups' mask-multiplies run on the Pool engine (emitted
# after the AllGathers have drained its queue), writing fp8 directly.
LN2 = 0.6931471805599453
SCL1 = 2.0 ** -6             # pm scale (fold into G)
SCLH = 2.0 ** -3             # hhat scale (fold into A)
DR_AFTER_L1 = 26             # groups (hp*16+mbp) >= this: fp8 DoubleRow + Pool
POOL_AFTER_L2 = 11           # of 16 L2 groups: mask-mult on Pool
PM_BUFS = 7
ZT_BUFS = 6
NO_COLL = False              # timing probe: skip AllGathers (results invalid)
NO_MASK = False              # timing probe: skip mask multiplies (results invalid)

bf16 = mybir.dt.bfloat16
f32 = mybir.dt.float32
fp8 = mybir.dt.float8e4
AF = mybir.ActivationFunctionType
ALU = mybir.AluOpType
AX = mybir.AxisListType


def _allgather(nc, in_d, out_d, nrows):
    if NO_COLL:
        # timing probe: fake the gather with a local row-copy (wrong data)
        nc.sync.dma_start(out_d[0:nrows, :], in_d[:])
        return
    nc.gpsimd.collective_compute(
        "AllGather", ALU.bypass,
        replica_groups=[list(range(NCORES))],
        ins=[in_d.opt()], outs=[out_d.opt()])


def _build(reps=1):
    nc = bacc.Bacc("TRN2", target_bir_lowering=False, debug=False,
                   num_devices=NCORES)

    xTq_d = nc.dram_tensor("xTq", [F, Q], bf16, kind="ExternalInput").ap()
    xTq8_d = nc.dram_tensor("xTq8", [F, Q], fp8, kind="ExternalInput").ap()
    w18_d = nc.dram_tensor("w18", [F, H * O], fp8, kind="ExternalInput").ap()
    maskT_d = nc.dram_tensor("maskT", [N, Q], bf16, kind="ExternalInput").ap()
    wsd_d = nc.dram_tensor("wsd", [F, 2 * H], bf16, kind="ExternalInput").ap()
    w2p_d = nc.dram_tensor("w2p", [O, PAY], bf16, kind="ExternalInput").ap()
    out_d = nc.dram_tensor("out", [Q, C], f32, kind="ExternalOutput").ap()

    with tile.TileContext(nc) as tc:
        for _ in range(reps):
            _emit(tc, xTq_d, xTq8_d, maskT_d, w18_d, wsd_d, w2p_d, out_d)
    nc.compile()
    return nc


def _emit(tc, xTq_d, xTq8_d, maskT_d, w18_d, wsd_d, w2p_d, out_d):
    nc = tc.nc
    with tc.tile_pool(name="singles", bufs=1) as singles:
        # ---- persistent SBUF tensors ----
        xTq_sb = singles.tile([P, KB, Q], bf16)
        xTq8_sb = singles.tile([P, KB, Q], fp8)
        w18_sb = singles.tile([P, KB, H * O], fp8)
        # (bf16 w1 dropped: the h projection runs fp8 DoubleRow)
        maskT_sb = singles.tile([P, MB, Q], bf16)
        wsd_sb = singles.tile([P, KB, 2 * H], bf16)
        w2p_sb = singles.tile([P, OB, PAY], bf16)
        ones1 = singles.tile([1, P], f32)
        ident = singles.tile([P, P], bf16)
        TH_sb = singles.tile([P, H, Q], bf16)     # exp(BETA*e_src) bcast
        Eown_sb = singles.tile([P, QC, 2 * H], f32)
        Aown_sb = singles.tile([P, H, QC], f32)   # exp(e_dst) own rows
        gpay_sb = singles.tile([P, QC, H], f32)   # exp(BETA*e_dst) own rows
        G_sb = singles.tile([P, MB, H], f32)      # gathered G, all nodes
        h_sbA = singles.tile([P, MB, HCH], bf16)  # heads 0-1 [hhat|A]
        h_sbB = singles.tile([P, MB, HCH], bf16)  # heads 2-3
        h8_sbB = singles.tile([P, MB, HCH], fp8)  # fp8 cast of late h_sbB
        hpayA_sb = singles.tile([P, QC, HCH], bf16)
        hpayB_sb = singles.tile([P, QC, HCH], bf16)
        x2acc = singles.tile([P, QC, O], f32)
        x2bf = singles.tile([P, QC, O], bf16)
        x2T = singles.tile([P, OB, Q], bf16)
        e2d_own = singles.tile([P, QC], f32)
        A2_sb = singles.tile([P, QC], f32)
        g2own_sb = singles.tile([P, QC], f32)
        pay_sb = singles.tile([P, QC, PAYG], bf16)
        h2g_sb = singles.tile([P, MB, PAYG], bf16)
        TH2_sb = singles.tile([P, Q], bf16)
        G2_all = singles.tile([P, MB], f32)

        # ---- input DMAs (order: smallest/earliest-needed first) ----
        xTq_r = xTq_d.rearrange("(kb p) q -> p kb q", p=P)
        for kb in range(KB):
            nc.sync.dma_start(xTq_sb[:, kb, :], xTq_r[:, kb, :])
        nc.sync.dma_start(xTq8_sb[:],
                          xTq8_d.rearrange("(kb p) q -> p kb q", p=P))
        nc.sync.dma_start(w18_sb[:],
                          w18_d.rearrange("(kb p) c -> p kb c", p=P))
        nc.sync.dma_start(wsd_sb[:], wsd_d.rearrange("(kb p) c -> p kb c", p=P))
        nc.sync.dma_start(w2p_sb[:], w2p_d.rearrange("(ob p) c -> p ob c", p=P))
        maskT_r = maskT_d.rearrange("(b p) q -> p b q", p=P)
        for g in range(8):
            s = bass.ts(g, MB // 8)
            nc.sync.dma_start(maskT_sb[:, s, :], maskT_r[:, s, :])

        nc.vector.memset(ones1[:], 1.0)
        bG = singles.tile([P, 1], f32)
        bA = singles.tile([P, 1], f32)
        nc.vector.memset(bG[:], -6.0 * LN2)   # G *= 2^-6
        nc.vector.memset(bA[:], -3.0 * LN2)   # A *= 2^-3
        make_identity(nc, ident[:])

        # ---- phase B: own-row E, tiny G gather, TH tiles ----
        with tc.tile_pool(name="b_psum", bufs=1, space="PSUM") as bpp, \
             tc.tile_pool(name="b_sb", bufs=2) as bsb, \
             tc.tile_pool(name="g_dram", bufs=1, space="DRAM") as gdram:
            for qc in range(QC):
                ps_E = bpp.tile([P, 2 * H], f32, tag="ps_E")
                for kb in range(KB):
                    nc.tensor.matmul(ps_E[:], xTq_sb[:, kb, bass.ts(qc, P)],
                                     wsd_sb[:, kb, :],
                                     start=(kb == 0), stop=(kb == KB - 1))
                nc.scalar.activation(Eown_sb[:, qc, :], ps_E[:], AF.Copy)
            for h in range(H):
                nc.scalar.activation(gpay_sb[:, :, h], Eown_sb[:, :, H + h],
                                     AF.Exp, scale=BETA, bias=bG[:])
            gpay_d = gdram.tile([Q, H], f32)
            ggath_d = gdram.tile([N, H], f32, addr_space="Shared")
            nc.sync.dma_start(gpay_d.rearrange("(qc p) c -> p qc c", p=P),
                              gpay_sb[:])
            _allgather(nc, gpay_d, ggath_d, Q)
            nc.sync.dma_start(G_sb[:],
                              ggath_d.rearrange("(b p) c -> p b c", p=P))
            for h in range(H):
                nc.scalar.activation(Aown_sb[:, h, :], Eown_sb[:, :, H + h],
                                     AF.Exp, bias=bA[:])
            # TH per head: esrcT row -> broadcast -> exp(BETA*.)
            for h in range(H):
                ps_es = bpp.tile([1, Q], f32, tag="ps_es")
                for kb in range(KB):
                    nc.tensor.matmul(ps_es[:], wsd_sb[:, kb, h:h + 1],
                                     xTq_sb[:, kb, :],
                                     start=(kb == 0), stop=(kb == KB - 1))
                esrcT = bsb.tile([1, Q], f32, tag="esrcT")
                nc.scalar.activation(esrcT[:], ps_es[:], AF.Copy)
                ps_b = bpp.tile([P, Q], f32, tag="ps_b")
                nc.tensor.matmul(ps_b[:], ones1[:], esrcT[:],
                                 start=True, stop=True)
                nc.scalar.activation(TH_sb[:, h, :], ps_b[:], AF.Exp,
                                     scale=BETA)

        # ---- phase C: hhat for own rows, 2-stage AllGather ----
        with tc.tile_pool(name="h_psum", bufs=2, space="PSUM") as hpp, \
             tc.tile_pool(name="h_dram", bufs=1, space="DRAM") as hdram:
            for qc in range(QC):
                ps_h = hpp.tile([P, H * O], f32, tag="ps_h")
                for half in range(2):
                    s = bass.ds(half * 512, 512)
                    for kp in range(KB // 2):
                        nc.tensor.matmul(
                            ps_h[:, s],
                            xTq8_sb[:, 2 * kp:2 * kp + 2, bass.ts(qc, P)],
                            w18_sb[:, 2 * kp:2 * kp + 2, s],
                            start=(kp == 0), stop=(kp == KB // 2 - 1),
                            perf_mode=mybir.MatmulPerfMode.DoubleRow)
                for h in range(H):
                    dst_sb = hpayA_sb if h < 2 else hpayB_sb
                    c0 = (h % 2) * HC
                    nc.scalar.activation(dst_sb[:, qc, c0:c0 + O],
                                         ps_h[:, bass.ts(h, O)], AF.Copy,
                                         scale=Aown_sb[:, h, qc:qc + 1])
                    nc.scalar.activation(dst_sb[:, qc, c0 + O:c0 + O + 1],
                                          Aown_sb[:, h, qc:qc + 1], AF.Copy)
            hpayA_d = hdram.tile([Q, HCH], bf16)
            hgathA_d = hdram.tile([N, HCH], bf16, addr_space="Shared")
            hpayB_d = hdram.tile([Q, HCH], bf16)
            hgathB_d = hdram.tile([N, HCH], bf16, addr_space="Shared")
            nc.sync.dma_start(hpayA_d.rearrange("(qc p) c -> p qc c", p=P),
                              hpayA_sb[:])
            nc.sync.dma_start(hpayB_d.rearrange("(qc p) c -> p qc c", p=P),
                              hpayB_sb[:])
            _allgather(nc, hpayA_d, hgathA_d, Q)
            _allgather(nc, hpayB_d, hgathB_d, Q)
            hgA_r = hgathA_d.rearrange("(b p) c -> p b c", p=P)
            hgB_r = hgathB_d.rearrange("(b p) c -> p b c", p=P)
            for g in range(8):
                s = bass.ts(g, MB // 8)
                nc.sync.dma_start(h_sbA[:, s, :], hgA_r[:, s, :])
                nc.sync.dma_start(h_sbB[:, s, :], hgB_r[:, s, :])

        # ---- phase D: layer-1 attention, head-pair outer ----
        with tc.tile_pool(name="acc_psum", bufs=1, space="PSUM") as accp, \
             tc.tile_pool(name="pm_pool", bufs=PM_BUFS) as pm_pool, \
             tc.tile_pool(name="pm8_pool", bufs=3) as pm8_pool, \
             tc.tile_pool(name="zt_pool", bufs=ZT_BUFS) as zt_pool, \
             tc.tile_pool(name="small1", bufs=4) as sp1:
            for hp in range(2):
                h_half = h_sbA if hp == 0 else h_sbB
                accs = {}
                for hh in range(2):
                    for qc in range(QC):
                        accs[(hh, qc)] = accp.tile(
                            [P, HC], f32, tag=f"acc{hh}_{qc}",
                            name=f"acc{hh}_{qc}")
                for mbp in range(MB // 2):
                    route_dr = (hp * (MB // 2) + mbp) >= DR_AFTER_L1
                    zt = zt_pool.tile([P, 2, 2, Q], bf16, tag="zt", name="zt")
                    for mbi in range(2):
                        mb = mbp * 2 + mbi
                        for hh in range(2):
                            h = hp * 2 + hh
                            # t = max(G*TH, 2^-6) on DVE (4x ts mode);
                            # the 2^-6 pm scale rides in G
                            nc.vector.tensor_scalar(
                                zt[:, mbi, hh, :], TH_sb[:, h, :],
                                G_sb[:, mb, h:h + 1], SCL1,
                                ALU.mult, ALU.max)
                    if route_dr:
                        # fp8 pm via Pool; values feed DoubleRow matmuls
                        pm8 = pm8_pool.tile([P, 2, 2, Q], fp8, tag="pm8",
                                            name="pm8")
                        for hh in range(2):
                            nc.gpsimd.tensor_mul(
                                pm8[:, :, hh, :], zt[:, :, hh, :],
                                maskT_sb[:, mbp * 2:mbp * 2 + 2, :])
                        nc.scalar.activation(
                            h8_sbB[:, mbp * 2:mbp * 2 + 2, :],
                            h_sbB[:, mbp * 2:mbp * 2 + 2, :], AF.Copy)
                        for hh in range(2):
                            rhs8 = h8_sbB[:, mbp * 2:mbp * 2 + 2,
                                          (hh * HC):(hh * HC + HC)]
                            for qc in range(QC):
                                nc.tensor.matmul(
                                    accs[(hh, qc)][:],
                                    pm8[:, :, hh, bass.ts(qc, P)],
                                    rhs8, start=False,
                                    stop=(mbp == MB // 2 - 1),
                                    perf_mode=mybir.MatmulPerfMode.DoubleRow)
                    else:
                        if NO_MASK:
                            pm = zt
                        else:
                            pm = pm_pool.tile([P, 2, 2, Q], bf16, tag="pm",
                                              name="pm")
                            for hh in range(2):
                                nc.vector.tensor_mul(
                                    pm[:, :, hh, :], zt[:, :, hh, :],
                                    maskT_sb[:, mbp * 2:mbp * 2 + 2, :])
                        for mbi in range(2):
                            mb = mbp * 2 + mbi
                            for hh in range(2):
                                rhs = h_half[:, mb, (hh * HC):(hh * HC + HC)]
                                for qc in range(QC):
                                    nc.tensor.matmul(
                                        accs[(hh, qc)][:],
                                        pm[:, mbi, hh, bass.ts(qc, P)],
                                        rhs, start=(mb == 0),
                                        stop=(mb == MB - 1))
                for hh in range(2):
                    h = hp * 2 + hh
                    for qc in range(QC):
                        r = sp1.tile([P, 1], f32, tag="r")
                        nc.vector.reciprocal(r[:], accs[(hh, qc)][:, O:O + 1])
                        nc.vector.tensor_scalar_mul(r[:], r[:], 1.0 / H)
                        if h == 0:
                            nc.scalar.activation(
                                x2acc[:, qc, :], accs[(hh, qc)][:, 0:O],
                                AF.Copy, scale=r[:])
                        else:
                            nc.vector.scalar_tensor_tensor(
                                x2acc[:, qc, :], accs[(hh, qc)][:, 0:O], r[:],
                                x2acc[:, qc, :], op0=ALU.mult, op1=ALU.add)

        # ---- phase E: relu, transpose, layer-2 projections ----
        nc.scalar.activation(x2bf[:], x2acc[:], AF.Relu)
        with tc.tile_pool(name="l2_psum", bufs=2, space="PSUM") as lpp, \
             tc.tile_pool(name="l2_sb", bufs=2) as lsb, \
             tc.tile_pool(name="g2_dram", bufs=1, space="DRAM") as g2dram:
            for qc in range(QC):
                for ob in range(OB):
                    tp = lpp.tile([P, P], bf16, tag="tp")
                    nc.tensor.transpose(tp[:], x2bf[:, qc, bass.ts(ob, P)],
                                        ident[:])
                    nc.scalar.activation(x2T[:, ob, bass.ts(qc, P)], tp[:],
                                         AF.Copy)
            for qc in range(QC):
                ps2 = lpp.tile([P, PAY], f32, tag="ps2")
                for ob in range(OB):
                    nc.tensor.matmul(ps2[:], x2T[:, ob, bass.ts(qc, P)],
                                     w2p_sb[:, ob, :],
                                     start=(ob == 0), stop=(ob == OB - 1))
                nc.scalar.activation(e2d_own[:, qc:qc + 1], ps2[:, C:C + 1],
                                      AF.Copy)
                nc.scalar.activation(A2_sb[:, qc:qc + 1], ps2[:, C:C + 1],
                                     AF.Exp)
                nc.scalar.activation(g2own_sb[:, qc:qc + 1], ps2[:, C:C + 1],
                                     AF.Exp, scale=BETA)
                nc.scalar.activation(pay_sb[:, qc, 0:C], ps2[:, 0:C], AF.Copy,
                                     scale=A2_sb[:, qc:qc + 1])
                nc.vector.tensor_copy(pay_sb[:, qc, C:C + 1],
                                      A2_sb[:, qc:qc + 1])
            # tiny G2 gather first, then the payload gather
            g2own_d = g2dram.tile([Q, 1], f32)
            g2gath_d = g2dram.tile([N, 1], f32, addr_space="Shared")
            nc.sync.dma_start(g2own_d.rearrange("(qc p) c -> p qc c", p=P),
                              g2own_sb[:, :, None])
            _allgather(nc, g2own_d, g2gath_d, Q)
            nc.sync.dma_start(G2_all[:, :, None],
                              g2gath_d.rearrange("(b p) c -> p b c", p=P))
            # TH2 from e2srcT
            ps_e2 = lpp.tile([1, Q], f32, tag="ps_e2")
            for ob in range(OB):
                nc.tensor.matmul(ps_e2[:], w2p_sb[:, ob, C + 1:C + 2],
                                 x2T[:, ob, :],
                                 start=(ob == 0), stop=(ob == OB - 1))
            e2srcT = lsb.tile([1, Q], f32, tag="e2srcT")
            nc.scalar.activation(e2srcT[:], ps_e2[:], AF.Copy)
            ps_b2 = lpp.tile([P, Q], f32, tag="ps_b2")
            nc.tensor.matmul(ps_b2[:], ones1[:], e2srcT[:],
                             start=True, stop=True)
            nc.scalar.activation(TH2_sb[:], ps_b2[:], AF.Exp, scale=BETA)

        # ---- phase F: AllGather packed payload ----
        with tc.tile_pool(name="dram", bufs=1, space="DRAM") as dram:
            pay_d = dram.tile([Q, PAYG], bf16)
            gath_d = dram.tile([N, PAYG], bf16, addr_space="Shared")
            nc.sync.dma_start(pay_d.rearrange("(qc p) c -> p qc c", p=P),
                              pay_sb[:])
            _allgather(nc, pay_d, gath_d, Q)
            h2g_r = gath_d.rearrange("(b p) c -> p b c", p=P)
            for g in range(4):
                s = bass.ts(g, MB // 4)
                nc.sync.dma_start(h2g_sb[:, s, :], h2g_r[:, s, :])

            # ---- phase G: layer-2 attention ----
            with tc.tile_pool(name="acc2_psum", bufs=1, space="PSUM") as acc2p, \
                 tc.tile_pool(name="pm2_pool", bufs=8) as pm2_pool, \
                 tc.tile_pool(name="zt2_pool", bufs=4) as zt2_pool, \
                 tc.tile_pool(name="small2", bufs=4) as sp2:
                accs2 = []
                for qc in range(QC):
                    accs2.append(acc2p.tile([P, C + 1], f32, tag=f"a2_{qc}",
                                            name=f"a2_{qc}"))
                for mbp in range(MB // 2):
                    route_pool = mbp >= POOL_AFTER_L2
                    zt2 = zt2_pool.tile([P, 2, Q], bf16, tag="zt2", name="zt2")
                    pm2 = pm2_pool.tile([P, 2, Q], bf16, tag="pm2", name="pm2")
                    for mbi in range(2):
                        mb = mbp * 2 + mbi
                        nc.vector.tensor_scalar(
                            zt2[:, mbi, :], TH2_sb[:],
                            G2_all[:, mb:mb + 1], 1.0,
                            ALU.mult, ALU.max)
                    if NO_MASK:
                        pm2 = zt2
                    else:
                        eng = nc.gpsimd if route_pool else nc.vector
                        eng.tensor_mul(
                            pm2[:], zt2[:],
                            maskT_sb[:, mbp * 2:mbp * 2 + 2, :])
                    for mbi in range(2):
                        mb = mbp * 2 + mbi
                        for qc in range(QC):
                            nc.tensor.matmul(accs2[qc][:],
                                             pm2[:, mbi, bass.ts(qc, P)],
                                             h2g_sb[:, mb, 0:C + 1],
                                             start=(mb == 0),
                                             stop=(mb == MB - 1))
                # log-softmax, batched by activation function
                logits_all = sp2.tile([P, QC, C], f32, tag="logits_all",
                                      name="logits_all")
                negmax_all = sp2.tile([P, QC], f32, tag="negmax_all",
                                      name="negmax_all")
                ssum_all = sp2.tile([P, QC], f32, tag="ssum_all",
                                    name="ssum_all")
                lse_all = sp2.tile([P, QC], f32, tag="lse_all", name="lse_all")
                for qc in range(QC):
                    r2 = sp2.tile([P, 1], f32, tag="r2")
                    nc.vector.reciprocal(r2[:], accs2[qc][:, C:C + 1])
                    nc.vector.tensor_scalar_mul(logits_all[:, qc, :],
                                                accs2[qc][:, 0:C], r2[:])
                    nc.vector.reduce_max(negmax_all[:, qc:qc + 1],
                                         logits_all[:, qc, :], axis=AX.X,
                                         negate=True)
                for qc in range(QC):
                    expt = sp2.tile([P, C], f32, tag="expt")
                    nc.scalar.activation(expt[:], logits_all[:, qc, :], AF.Exp,
                                         bias=negmax_all[:, qc:qc + 1],
                                         accum_out=ssum_all[:, qc:qc + 1])
                nc.scalar.activation(lse_all[:], ssum_all[:], AF.Ln)
                for qc in range(QC):
                    res = sp2.tile([P, C], f32, tag="res")
                    nc.vector.tensor_scalar(res[:], logits_all[:, qc, :],
                                            negmax_all[:, qc:qc + 1],
                                            lse_all[:, qc:qc + 1],
                                            ALU.add, ALU.subtract)
                    nc.sync.dma_start(out_d[bass.ts(qc, P), :], res[:])


_CACHED = None


def _get_nc():
    global _CACHED
    if _CACHED is None:
        _CACHED = _build()
    return _CACHED


def kernel(x, adj, W1, a1, W2, a2):
    x = np.asarray(x, dtype=np.float32)
    adj = np.asarray(adj)
    W1 = np.asarray(W1, dtype=np.float32)
    a1 = np.asarray(a1, dtype=np.float32)
    W2 = np.asarray(W2, dtype=np.float32)
    a2 = np.asarray(a2, dtype=np.float32)

    bf = ml_dtypes.bfloat16
    f8 = ml_dtypes.float8_e4m3
    xT = np.ascontiguousarray(x.T).astype(bf)                     # [F, N]
    # fused score weights: e_src = x @ (W1 @ a_src), e_dst likewise
    wsrc = np.einsum("hfo,ho->fh", W1, a1[:, :O])                 # [F, H]
    wdst = np.einsum("hfo,ho->fh", W1, a1[:, O:])                 # [F, H]
    wsd = np.concatenate([wsrc, wdst], axis=1).astype(bf)         # [F, 2H]
    w1cat = np.concatenate([W1[h] for h in range(H)], 1).astype(bf)  # [F, H*O]
    w2p = np.zeros((O, PAY), np.float32)
    w2p[:, 0:C] = W2[0]
    w2p[:, C] = W2[0] @ a2[0, C:]      # e2_dst vector
    w2p[:, C + 1] = W2[0] @ a2[0, :C]  # e2_src vector
    w2p = w2p.astype(bf)

    adj_on = adj > 0
    in_maps = []
    for c in range(NCORES):
        rows = slice(c * Q, (c + 1) * Q)
        in_maps.append({
            "xTq": np.ascontiguousarray(xT[:, rows]),
            "xTq8": np.ascontiguousarray(xT[:, rows]).astype(f8),
            "maskT": np.ascontiguousarray(adj_on[rows, :].T).astype(bf),
            "w18": w1cat.astype(f8),
            "wsd": wsd,
            "w2p": w2p,
        })

    nc = _get_nc()
    res = run_bass_kernel_spmd(nc, in_maps, core_ids=list(range(NCORES)))
    return np.concatenate([res.results[c]["out"] for c in range(NCORES)], 0)
